# revision 37
# baseline (speedup 1.0000x reference)
"""FaceAttnProcessor Trainium2 kernel (v3).

Sharding: 8 cores = batch(2) x row-slices(4 x 256 rows). Each core computes
its 256 query rows end-to-end (self-attn with redundant K/V over the full
1040-token sequence, GEGLU FF, cross-attn against the 77 text tokens).
No collectives; the host scatters inputs and gathers the 8 row-slices.

Layout/schedule:
- Host pre-packs all weights into bf16 blobs already in SBUF layout, so
  every weight DMA is a straight slice copy with multi-KB descriptors
  (halves the weight traffic vs fp32, no on-device rearranges).
- Host permutes x_full so the core's own 256 rows come first: the Q
  source is cT[:, :, 0:256] (no separate x_own load / LN).
- All matmuls in bf16 (1 PE cycle/row at any free size, fp32 PSUM
  accumulation). LN outputs cast to bf16 at the normalize step so the
  PE transposes run at 1 cycle/row too.
- SA softmax row-sums are free: V carries a ones-column (col 64 of each
  head block), so the attnV matmul's output row 64 is the denominator.
  Reciprocals are broadcast across partitions with a 1-row PE matmul.
- CA is shift-free: head1's attnV writes PSUM partitions 64:128 directly,
  row-sums come from one ones-vector matmul over both heads' exp tiles.
- SA pipeline runs scores(hp+1) before attnV(hp) so the Act-engine exp
  for hp completes while the PE scores hp+1 (no est-wait bubbles).
- Weight stream (Pool/SWDGE queue) in consumption order from t=0;
  wbig closes right after QKV so the FF weight pools alias its space and
  their DMAs only wait for the QKV matmuls, streaming during attention.
"""
import numpy as np
from contextlib import ExitStack

import concourse.bass as bass
import concourse.tile as tile
import concourse.mybir as mybir
from concourse import bacc
from concourse.bass_utils import run_bass_kernel_spmd
from concourse.masks import make_identity

F32 = mybir.dt.float32
F32R = mybir.dt.float32r
BF16 = mybir.dt.bfloat16
AFT = mybir.ActivationFunctionType

P = 128
B, N, C, L = 2, 1024, 768, 93
NT, NF = 77, 16            # text / face tokens
NTP = 80                   # text tokens padded
NC_ = 1040                 # N + NF combined sequence
R = 256                    # query rows per core
H, D = 12, 64              # heads, head dim
HP = 6                     # head pairs
INNER = 3072
KC = 6                     # C // 128
EPS = 1e-5

_cache = {}


def build(fast_ln2=False):
    nc = bacc.Bacc("TRN2", target_bir_lowering=False, debug=False, num_devices=8)

    x_full_d = nc.dram_tensor("x_full", [N, C], F32, kind="ExternalInput")
    xb_d = nc.dram_tensor("xb", [P, 8, C], BF16, kind="ExternalInput")
    face_d = nc.dram_tensor("face", [NF, C], F32, kind="ExternalInput")
    ehsT_d = nc.dram_tensor("ehsT", [P, KC, NTP], BF16, kind="ExternalInput")
    lnvT_d = nc.dram_tensor("lnvT", [P, KC, 4], F32, kind="ExternalInput")
    bcast_d = nc.dram_tensor("bcast", [P, 4, C], F32, kind="ExternalInput")
    alph_d = nc.dram_tensor("alph", [1, 2], F32, kind="ExternalInput")
    wv_d = nc.dram_tensor("wv", [P, KC, C], BF16, kind="ExternalInput")
    wk_d = nc.dram_tensor("wk", [P, KC, C], BF16, kind="ExternalInput")
    wq_d = nc.dram_tensor("wq", [P, KC, C], BF16, kind="ExternalInput")
    ck_d = nc.dram_tensor("ck", [P, KC, C], BF16, kind="ExternalInput")
    cv_d = nc.dram_tensor("cv", [P, KC, C], BF16, kind="ExternalInput")
    wo_d = nc.dram_tensor("wo", [D, H, C], BF16, kind="ExternalInput")
    w1_d = nc.dram_tensor("w1", [P, 12, KC, 2, 256], BF16, kind="ExternalInput")
    w2_d = nc.dram_tensor("w2", [P, 24, C], BF16, kind="ExternalInput")
    cq_d = nc.dram_tensor("cq", [P, KC, C], BF16, kind="ExternalInput")
    co_d = nc.dram_tensor("co", [P, HP, C], BF16, kind="ExternalInput")
    out_d = nc.dram_tensor("out_own", [R, C], F32, kind="ExternalOutput")

    with tile.TileContext(nc) as tc, ExitStack() as ctx:
        consts = ctx.enter_context(tc.tile_pool(name="consts", bufs=1))
        acts = ctx.enter_context(tc.tile_pool(name="acts", bufs=1))
        tmp1 = ctx.enter_context(tc.tile_pool(name="tmp1", bufs=1))
        tmp = ctx.enter_context(tc.tile_pool(name="tmp", bufs=2))
        dram = ctx.enter_context(tc.tile_pool(name="dram", bufs=1, space="DRAM"))

        # ---------------- input loads (SP queue): critical-path first ------
        xf = acts.tile([P, 8, C], BF16, tag="xf")
        nc.sync.dma_start(xf[:, 0, :], xb_d[:, 0, :])
        alo = consts.tile([1, 2], F32)
        nc.sync.dma_start(alo[:], alph_d[:])
        lnvT = consts.tile([P, KC, 4], F32, tag="lnvT")
        nc.sync.dma_start(lnvT[:], lnvT_d[:])
        for rc in range(1, 8):
            nc.sync.dma_start(xf[:, rc, :], xb_d[:, rc, :])
        face = consts.tile([NF, C], F32, tag="face")
        nc.sync.dma_start(face[:], face_d[:])
        ehsT = consts.tile([P, KC, NTP], BF16, tag="ehsT")
        nc.sync.dma_start(ehsT[:], ehsT_d[:])

        # tanh(alpha) -> [128, 1] per-partition broadcast via DRAM roundtrip
        th = consts.tile([1, 2], F32)
        nc.scalar.activation(th[:], alo[:], AFT.Tanh)
        tanh_dr = dram.tile([1, 2], F32)
        nc.sync.dma_start(tanh_dr[:], th[:])
        tA = consts.tile([P, 1], F32, tag="tA")
        nc.sync.dma_start(tA[:], tanh_dr[0:1, 0:1].to_broadcast([P, 1]))
        tD = consts.tile([P, 1], F32, tag="tD")
        nc.sync.dma_start(tD[:], tanh_dr[0:1, 1:2].to_broadcast([P, 1]))
        obias = consts.tile([P, 2, C], F32, tag="obias")   # {sa_wo_b, ca_wo_b}
        nc.sync.dma_start(obias[:], bcast_d[:, 2:4, :])
        xo = acts.tile([P, 2, C], F32, tag="xo")
        nc.sync.dma_start(xo[:], x_full_d[0:R, :].rearrange(
            "(rc p) c -> p rc c", p=P))

        eps_t = consts.tile([P, 1], F32)
        nc.vector.memset(eps_t[:], EPS)
        actwarm = consts.tile([1, 6], F32)
        nc.scalar.activation(actwarm[:, 0:1], th[0:1, 0:1], AFT.Sqrt,
                             scale=0.0, bias=eps_t[0:1, 0:1])
        ones_r = consts.tile([1, P], F32R)
        nc.vector.memset(ones_r[:].bitcast(F32), 1.0)

        wobB, cobB = obias[:, 0, :], obias[:, 1, :]

        # ---------------- helpers ----------------
        def ln_stats(x_ap, p):
            """Normalized (x-m)/std of x_ap [p, 768], cast to bf16.
            Square-sum on Act; mean-sum on DVE (engine balance)."""
            junk = tmp1.tile([P, C], F32, tag="ln_j")
            vsum = tmp.tile([P, 1], F32, tag="ln_vs")
            nc.scalar.activation(junk[:p], x_ap, AFT.Square, accum_out=vsum[:p])
            mean = tmp.tile([P, 1], F32, tag="ln_mean")
            nc.vector.reduce_sum(mean[:p], x_ap, axis=mybir.AxisListType.X)
            nc.vector.tensor_scalar_mul(mean[:p], mean[:p], 1.0 / C)
            m2 = tmp.tile([P, 1], F32, tag="ln_m2")
            nc.vector.tensor_mul(m2[:p], mean[:p], mean[:p])
            var = tmp.tile([P, 1], F32, tag="ln_var")
            nc.vector.tensor_scalar_mul(var[:p], vsum[:p], 1.0 / C)
            nc.vector.tensor_sub(var[:p], var[:p], m2[:p])
            std = tmp.tile([P, 1], F32, tag="ln_std")
            nc.scalar.activation(std[:p], var[:p], AFT.Sqrt, bias=eps_t[:p, 0:1])
            rstd = tmp.tile([P, 1], F32, tag="ln_rstd")
            nc.vector.reciprocal(rstd[:p], std[:p])
            xn = tmp.tile([P, C], BF16, tag="ln_xnb")
            nc.vector.tensor_scalar(xn[:p], x_ap, mean[:p], rstd[:p],
                                    mybir.AluOpType.subtract, mybir.AluOpType.mult)
            return xn

        def transpose_gb(ps_t, xn, p, dst, col, gi, bi, flip=0):
            """PE-transpose bf16 xn [p,768] into dst[:, k, col:col+p] (bf16),
            applying per-channel gain lnvT[:,k,gi] / bias lnvT[:,k,bi]."""
            for k in range(KC):
                pt = ps_t.tile([P, P], BF16, tag="tp")
                nc.tensor.transpose(pt[:, 0:p], xn[:p, bass.ts(k, P)],
                                    identB[:p, :p])
                if (k + flip) % 2 == 0:
                    nc.vector.tensor_scalar(
                        dst[:, k, col:col + p], pt[:, 0:p],
                        lnvT[:, k, gi:gi + 1], lnvT[:, k, bi:bi + 1],
                        mybir.AluOpType.mult, mybir.AluOpType.add)
                else:
                    nc.scalar.activation(
                        dst[:, k, col:col + p], pt[:, 0:p],
                        AFT.Identity, bias=lnvT[:, k, bi:bi + 1],
                        scale=lnvT[:, k, gi:gi + 1])

        # ---------------- persistent activations ----------------
        x1 = acts.tile([P, 2, C], F32, tag="x1")
        x2 = acts.tile([P, 2, C], F32, tag="x2")
        KcaT = acts.tile([P, KC, NTP], BF16, tag="KcaT")
        Vca = acts.tile([NTP, H, D + 1], BF16, tag="Vca")

        with tc.tile_pool(name="saout", bufs=1) as saout:
            attnU = saout.tile([D, H, R], BF16, tag="attnU")
            QT = saout.tile([P, KC, R], BF16, tag="QT")
            KT = saout.tile([P, KC, NC_], BF16, tag="KT")
            V = saout.tile([P, 9, H, D + 1], BF16, tag="V")
            wot = saout.tile([D, H, C], BF16, tag="wot")

            with tc.tile_pool(name="wbig", bufs=1) as wbig:
                # weight stream, consumption order (Pool/SWDGE queue)
                # wv/wk/wq in 2-ko chunks so the bf16 x loads interleave
                # on the DMA engines instead of stalling behind 3.3us blocks
                wvt = wbig.tile([P, KC, C], BF16, tag="wvt")
                nc.gpsimd.dma_start(wvt[:, 0:2, :], wv_d[:, 0:2, :])
                identB = consts.tile([P, P], BF16)
                make_identity(nc, identB[:])      # gpsimd memset+affine_select
                identF = consts.tile([P, P], F32)
                make_identity(nc, identF[:])
                nc.gpsimd.dma_start(wvt[:, 2:4, :], wv_d[:, 2:4, :])
                nc.gpsimd.dma_start(wvt[:, 4:6, :], wv_d[:, 4:6, :])
                wkt = wbig.tile([P, KC, C], BF16, tag="wkt")
                for j in range(3):
                    nc.gpsimd.dma_start(wkt[:, 2 * j:2 * j + 2, :],
                                        wk_d[:, 2 * j:2 * j + 2, :])
                wqt = wbig.tile([P, KC, C], BF16, tag="wqt")
                for j in range(3):
                    nc.gpsimd.dma_start(wqt[:, 2 * j:2 * j + 2, :],
                                        wq_d[:, 2 * j:2 * j + 2, :])
                ckt = wbig.tile([P, KC, C], BF16, tag="ckt")
                nc.gpsimd.dma_start(ckt[:], ck_d[:])
                cvt = wbig.tile([P, KC, C], BF16, tag="cvt")
                nc.gpsimd.dma_start(cvt[:], cv_d[:])
                nc.gpsimd.dma_start(wot[:], wo_d[:])
                nc.gpsimd.memset(V[:, :, :, D:D + 1], 1.0)
                nc.gpsimd.memset(Vca[:, :, D:D + 1], 1.0)

                with tc.tile_pool(name="pre", bufs=1) as pre, \
                     tc.tile_pool(name="ps_t0", bufs=3, space="PSUM") as ps_t0, \
                     tc.tile_pool(name="ps_qkv", bufs=3, space="PSUM") as ps_qkv:
                    cT = pre.tile([P, KC, NC_], BF16, tag="cT")

                    # warmup transpose (first real one carries a sem wait)
                    ptw = ps_t0.tile([P, P], BF16, tag="tp")
                    nc.tensor.transpose(ptw[:], identB[:], identB[:])

                    def v_chunk(rc, p):
                        for f0, fw, h0, nh in ((0, 512, 0, 8), (512, 256, 8, 4)):
                            pv = ps_qkv.tile([P, 512], F32, tag="pqkv", name="pv")
                            for k in range(KC):
                                nc.tensor.matmul(pv[:p, 0:fw],
                                                 cT[:, k, rc * P:rc * P + p],
                                                 wvt[:, k, f0:f0 + fw],
                                                 start=(k == 0),
                                                 stop=(k == KC - 1))
                            src = pv[:p, 0:fw].rearrange("p (a b) -> p a b", a=nh)
                            if rc % 3 == 2:
                                nc.scalar.activation(V[:p, rc, h0:h0 + nh, 0:D],
                                                     src, AFT.Identity)
                            else:
                                nc.vector.tensor_copy(V[:p, rc, h0:h0 + nh, 0:D],
                                                      src)

                    for rc in range(8):
                        xn = ln_stats(xf[:, rc, :], P)
                        transpose_gb(ps_t0, xn, P, cT, rc * P, 0, 1, rc)
                        v_chunk(rc, P)
                    fn = ln_stats(face[:], NF)
                    transpose_gb(ps_t0, fn, NF, cT, N, 0, 1)
                    v_chunk(8, NF)

                    # Q^T (scale 1/8 folded), DVE copyback
                    for f in range(KC):
                        pq = ps_qkv.tile([P, 512], F32, tag="pqkv", name="pq")
                        for k in range(KC):
                            nc.tensor.matmul(pq[:, 0:R],
                                             wqt[:, k, bass.ts(f, P)],
                                             cT[:, k, 0:R],
                                             start=(k == 0), stop=(k == KC - 1))
                        nc.vector.tensor_scalar_mul(QT[:, f, :], pq[:, 0:R],
                                                    0.125)

                    # K^T in 512-token chunks (copyback mostly DVE)
                    for f in range(KC):
                        for j0, jw in ((0, 512), (512, 512), (1024, NF)):
                            pk = ps_qkv.tile([P, 512], F32, tag="pqkv", name="pk")
                            for k in range(KC):
                                nc.tensor.matmul(pk[:, 0:jw],
                                                 wkt[:, k, bass.ts(f, P)],
                                                 cT[:, k, j0:j0 + jw],
                                                 start=(k == 0),
                                                 stop=(k == KC - 1))
                            if f % 3 == 2:
                                nc.scalar.activation(KT[:, f, j0:j0 + jw],
                                                     pk[:, 0:jw], AFT.Identity)
                            else:
                                nc.vector.tensor_copy(KT[:, f, j0:j0 + jw],
                                                      pk[:, 0:jw])

                    # CA K^T and V_ca (text only)
                    for f in range(KC):
                        pk = ps_qkv.tile([P, 512], F32, tag="pqkv", name="pck")
                        for k in range(KC):
                            nc.tensor.matmul(pk[:, 0:NTP],
                                             ckt[:, k, bass.ts(f, P)],
                                             ehsT[:, k, :],
                                             start=(k == 0), stop=(k == KC - 1))
                        if f % 2 == 0:
                            nc.vector.tensor_copy(KcaT[:, f, :], pk[:, 0:NTP])
                        else:
                            nc.scalar.activation(KcaT[:, f, :], pk[:, 0:NTP],
                                                 AFT.Identity)
                    for f0, fw, h0, nh in ((0, 512, 0, 8), (512, 256, 8, 4)):
                        pv = ps_qkv.tile([P, 512], F32, tag="pqkv", name="pcv")
                        for k in range(KC):
                            nc.tensor.matmul(pv[0:NTP, 0:fw], ehsT[:, k, :],
                                             cvt[:, k, f0:f0 + fw],
                                             start=(k == 0), stop=(k == KC - 1))
                        src = pv[0:NTP, 0:fw].rearrange("p (a b) -> p a b", a=nh)
                        nc.vector.tensor_copy(Vca[:, h0:h0 + nh, 0:D], src)

            # wbig closed: FF weight pools alias its space; their DMAs only
            # wait for the QKV matmuls, so w1/w2 stream during attention.
            with tc.tile_pool(name="wff1", bufs=3) as wff1, \
                 tc.tile_pool(name="wff2", bufs=4) as wff2:
                w1cs, w2cs = [], []
                for fc in range(12):
                    if fc % 3 == 0:
                        w2c = wff2.tile([P, KC, C], BF16, tag="w2c",
                                        name=f"w2c{fc // 3}")
                        nc.gpsimd.dma_start(
                            w2c[:], w2_d[:, (fc // 3) * KC:(fc // 3 + 1) * KC, :])
                        w2cs.append(w2c)
                    w1c = wff1.tile([P, KC, 2, 256], BF16, tag="w1c",
                                    name=f"w1c{fc}")
                    nc.gpsimd.dma_start(w1c[:], w1_d[:, fc, :, :, :])
                    w1cs.append(w1c)

                # x1 base = x + tanh(aa)*wo_b, on gpsimd: the obias/xo
                # DMAs land late and these would head-of-line block the DVE
                wobt = tmp1.tile([P, C], F32, tag="wobt")
                nc.gpsimd.tensor_scalar_mul(wobt[:], wobB, tA[:, 0:1])
                for qc in range(2):
                    nc.gpsimd.tensor_add(x1[:, qc, :], xo[:, qc, :], wobt[:])

                # pre-pull the exp act table while the last K chunks run
                nc.scalar.activation(actwarm[:, 4:5], KT[0:1, KC - 1, NC_ - 1:NC_],
                                     AFT.Exp)

                # ---- self-attention: scores(hp+1) issued before attnV(hp) --
                with tc.tile_pool(name="ps_sc", bufs=2, space="PSUM") as ps_sc, \
                     tc.tile_pool(name="ps_av", bufs=1, space="PSUM") as ps_av, \
                     tc.tile_pool(name="ps_pb", bufs=1, space="PSUM") as ps_pb, \
                     tc.tile_pool(name="ps_po", bufs=2, space="PSUM") as ps_po, \
                     tc.tile_pool(name="expp", bufs=10) as expp:
                    ests_all, pavs, pbs, rss = {}, {}, {}, {}

                    def sa_scores(hp):
                        # two rc tiles share one 2-bank psc and one exp call
                        # (fewer Act instructions; Act is the attention limit)
                        ests = []
                        for pair in range(5):
                            rcs = [r for r in (2 * pair, 2 * pair + 1) if r < 9]
                            nsl = 2 * len(rcs)
                            psc = ps_sc.tile([P, 4, R], F32, tag="psc")
                            est = expp.tile([P, 4, R], BF16, tag="est",
                                            name=f"est{hp}_{pair}")
                            for j, rc in enumerate(rcs):
                                p = P if rc < 8 else NF
                                ests.append((est, 2 * j))
                                for h01 in range(2):
                                    nc.tensor.matmul(
                                        psc[0:p, 2 * j + h01, :],
                                        KT[h01 * D:(h01 + 1) * D, hp,
                                           rc * P:rc * P + p],
                                        QT[h01 * D:(h01 + 1) * D, hp, :],
                                        start=True, stop=True)
                            p = P if rcs[-1] < 8 else NF
                            if p == P:
                                nc.scalar.activation(est[:, 0:nsl, :],
                                                     psc[:, 0:nsl, :], AFT.Exp)
                            else:
                                nc.scalar.activation(est[0:p, 0:nsl, :],
                                                     psc[0:p, 0:nsl, :],
                                                     AFT.Exp)
                        ests_all[hp] = ests

                    def sa_attnv(hp):
                        # sequential accumulation groups (A then B): two open
                        # groups may not share a 2KB PSUM zero region
                        ests = ests_all[hp]
                        pav = ps_av.tile([P, 2, R], F32, tag="pav",
                                         name=f"pav{hp}")
                        pavA, pavB = pav[:, 0, :], pav[:, 1, :]
                        for h01 in range(2):
                            dst = pavA if h01 == 0 else pavB
                            for rc in range(9):
                                p = P if rc < 8 else NF
                                et, sl = ests[rc]
                                nc.tensor.matmul(dst[0:D + 1, :],
                                                 V[0:p, rc, 2 * hp + h01, :],
                                                 et[0:p, sl + h01, :],
                                                 start=(rc == 0), stop=(rc == 8))
                        rs = tmp.tile([1, 2, R], F32R, tag="rs", name=f"rs{hp}")
                        with nc.allow_low_precision(reason="f32r softmax recip"):
                            nc.vector.reciprocal(rs[:, 0, :], pavA[D:D + 1, :])
                            nc.vector.reciprocal(rs[:, 1, :], pavB[D:D + 1, :])
                        pavs[hp] = (pavA, pavB)
                        rss[hp] = rs

                    def sa_bcast(hp):
                        pb = ps_pb.tile([D, 2 * R], F32, tag="pb", name=f"pb{hp}")
                        nc.tensor.matmul(pb[:], ones_r[0:1, 0:D],
                                         rss[hp][:].rearrange("p a b -> p (a b)"),
                                         start=True, stop=True)
                        # DVE may read only one PSUM operand per instruction:
                        # stage the broadcast reciprocals in SBUF
                        pbs_sb = tmp.tile([D, 2 * R], F32, tag="pbs",
                                          name=f"pbs{hp}")
                        nc.vector.tensor_copy(pbs_sb[:], pb[:])
                        pbs[hp] = pbs_sb

                    def sa_divide(hp):
                        pavA, pavB = pavs[hp]
                        pb = pbs[hp]
                        nc.vector.tensor_mul(attnU[0:D, 2 * hp, :], pavA[0:D, :],
                                             pb[:, 0:R])
                        nc.vector.tensor_mul(attnU[0:D, 2 * hp + 1, :],
                                             pavB[0:D, :], pb[:, R:2 * R])

                    poq = [ps_po.tile([P, 384], F32, tag="poq",
                                      name=f"poq{j}") for j in range(2)]

                    def sa_oproj_step(hp):
                        # qc0 half of the O-proj, folded into the attention
                        # loop; single-head 64-contraction steps avoid any
                        # partition-shift of the attention output
                        for j in range(2):
                            for h01 in range(2):
                                h = 2 * hp + h01
                                nc.tensor.matmul(
                                    poq[j][:], attnU[0:D, h, 0:P],
                                    wot[:, h, 384 * j:384 * (j + 1)],
                                    start=(h == 0), stop=(h == H - 1))

                    sa_scores(0)
                    sa_scores(1)
                    sa_attnv(0)
                    for hp in range(2, HP):
                        sa_bcast(hp - 2)
                        sa_divide(hp - 2)
                        sa_attnv(hp - 1)
                        sa_oproj_step(hp - 2)
                        sa_scores(hp)
                    sa_bcast(HP - 2)
                    sa_divide(HP - 2)
                    sa_attnv(HP - 1)
                    sa_oproj_step(HP - 2)
                    sa_bcast(HP - 1)
                    sa_divide(HP - 1)
                    sa_oproj_step(HP - 1)
                    nc.scalar.activation(actwarm[:, 1:2],
                                         attnU[0:1, H - 1, 0:1], AFT.Sqrt,
                                         scale=0.0, bias=eps_t[0:1, 0:1])
                    # x1 qc0 while still inside the attention pools
                    for j in range(2):
                        t = tmp.tile([P, 512], F32, tag="pot")
                        nc.scalar.activation(t[:, 0:384], poq[j][:], AFT.Copy,
                                             scale=tA[:, 0:1])
                        nc.vector.tensor_add(x1[:, 0, 384 * j:384 * (j + 1)],
                                             x1[:, 0, 384 * j:384 * (j + 1)],
                                             t[:, 0:384])

                # ---- O-proj qc1 + gated residual -> x1 ----
                with tc.tile_pool(name="ps_pr", bufs=2, space="PSUM") as ps_pr:
                    for f0, fw in ((0, 384), (384, 384)):
                        po = ps_pr.tile([P, 384], F32, tag="po")
                        for h in range(H):
                            nc.tensor.matmul(po[:],
                                             attnU[0:D, h, P:2 * P],
                                             wot[:, h, f0:f0 + fw],
                                             start=(h == 0),
                                             stop=(h == H - 1))
                        t = tmp.tile([P, 512], F32, tag="pot")
                        nc.scalar.activation(t[:, 0:fw], po[:], AFT.Copy,
                                             scale=tA[:, 0:1])
                        nc.vector.tensor_add(x1[:, 1, f0:f0 + fw],
                                             x1[:, 1, f0:f0 + fw],
                                             t[:, 0:fw])

                # ---------------- FF ----------------
                with tc.tile_pool(name="ffp", bufs=1) as ffp, \
                     tc.tile_pool(name="ps_tf", bufs=2, space="PSUM") as ps_tf:
                    hT = ffp.tile([P, KC, R], BF16, tag="hT")
                    if fast_ln2:
                        # ln2_g == 1, ln2_b == 0: LN(LN(x)) == LN(x) up to
                        # O(eps) ~ 5e-6 -- skip the second stats pass
                        for rc in range(2):
                            xn = ln_stats(x1[:, rc, :], P)
                            transpose_gb(ps_tf, xn, P, hT, rc * P, 2, 3, rc)
                    else:
                        g2b = ffp.tile([P, 2, C], F32, tag="g2b")
                        nc.sync.dma_start(g2b[:], bcast_d[:, 0:2, :])
                        for rc in range(2):
                            xn = ln_stats(x1[:, rc, :], P)
                            y = tmp1.tile([P, C], BF16, tag="ffy")
                            nc.vector.tensor_mul(y[:], xn[:], g2b[:, 0, :])
                            nc.vector.tensor_add(y[:], y[:], g2b[:, 1, :])
                            zn = ln_stats(y[:], P)
                            transpose_gb(ps_tf, zn, P, hT, rc * P, 2, 3, rc)

                    nc.scalar.activation(actwarm[:, 2:3], hT[0:1, KC - 1, R - 1:R],
                                         AFT.Gelu)
                    actT = ffp.tile([P, 24, R], BF16, tag="actT")
                    ffTb = ffp.tile([P, KC, R], BF16, tag="ffTb")
                    with tc.tile_pool(name="ps_h1", bufs=2,
                                      space="PSUM") as ps_h1:
                        for fc in range(12):
                            w1c = w1cs[fc]
                            for fi in range(2):
                                ft = fc * 2 + fi
                                pag = ps_h1.tile([P, 2, R], F32, tag="ph1",
                                                 name="pag")
                                pa, pg = pag[:, 0, :], pag[:, 1, :]
                                for k in range(KC):
                                    nc.tensor.matmul(
                                        pa[:], w1c[:, k, 0, bass.ts(fi, P)],
                                        hT[:, k, :],
                                        start=(k == 0), stop=(k == KC - 1))
                                for k in range(KC):
                                    nc.tensor.matmul(
                                        pg[:], w1c[:, k, 1, bass.ts(fi, P)],
                                        hT[:, k, :],
                                        start=(k == 0), stop=(k == KC - 1))
                                gl = tmp.tile([P, R], F32, tag="gl")
                                nc.scalar.activation(gl[:], pg[:], AFT.Gelu)
                                nc.vector.tensor_mul(actT[:, ft, :], pa[:],
                                                     gl[:])

                    # FF2: f-outer so each f's 24-matmul chain completes
                    # before the next (no two open groups per PSUM bank)
                    with tc.tile_pool(name="ps_f2", bufs=3,
                                      space="PSUM") as ps_f2:
                        pf2 = [ps_f2.tile([P, 2, R], F32, tag="pf",
                                          name=f"pf{j}") for j in range(3)]
                        pfs = [pf2[f // 2][:, f % 2, :] for f in range(KC)]
                        for f in range(KC):
                            for qb in range(4):
                                for k in range(KC):
                                    nc.tensor.matmul(
                                        pfs[f][:],
                                        w2cs[qb][:, k, bass.ts(f, P)],
                                        actT[:, qb * KC + k, :],
                                        start=(qb == 0 and k == 0),
                                        stop=(qb == 3 and k == KC - 1))
                            # tanh(ad) folded in; bf16 for cheap transposes
                            nc.scalar.activation(ffTb[:, f, :], pfs[f][:],
                                                 AFT.Copy, scale=tD[:, 0:1])

                    # x2 = x1 + ff^T (already tanh(ad)-scaled)
                    for qc in range(2):
                        for k in range(KC):
                            pt = ps_tf.tile([P, P], BF16, tag="tp")
                            nc.tensor.transpose(pt[:], ffTb[:, k, bass.ts(qc, P)],
                                                identB[:])
                            nc.vector.tensor_add(x2[:, qc, bass.ts(k, P)], pt[:],
                                                 x1[:, qc, bass.ts(k, P)])

        # ---------------- cross-attention (shift-free) ----------------
        with tc.tile_pool(name="cap", bufs=1) as cap:
            nc.scalar.activation(actwarm[:, 3:4], x2[0:1, 1, C - 1:C],
                                 AFT.Exp)
            x2T = cap.tile([P, KC, R], BF16, tag="x2T")
            with tc.tile_pool(name="ps_tc", bufs=4, space="PSUM") as ps_tc:
                for k in range(KC):
                    for qc in range(2):
                        pt = ps_tc.tile([P, P], F32, tag="tpc")
                        nc.tensor.transpose(pt[:], x2[:, qc, bass.ts(k, P)],
                                            identF[:])
                        if (2 * k + qc) % 3 == 0:
                            nc.vector.tensor_copy(x2T[:, k, bass.ts(qc, P)],
                                                  pt[:])
                        else:
                            nc.scalar.activation(x2T[:, k, bass.ts(qc, P)],
                                                 pt[:], AFT.Identity)

            x2c = cap.tile([P, 2, C], F32, tag="x2c")
            for qc in range(2):
                nc.vector.tensor_add(x2c[:, qc, :], x2[:, qc, :], cobB[:])
            qcaT = cap.tile([P, KC, R], BF16, tag="qcaT")
            with tc.tile_pool(name="wstr3", bufs=1) as wstr3:
                cqt = wstr3.tile([P, KC, C], BF16, tag="cqt")
                nc.gpsimd.dma_start(cqt[:], cq_d[:])
                cot = wstr3.tile([P, HP, C], BF16, tag="cot")
                nc.gpsimd.dma_start(cot[:], co_d[:])
                with tc.tile_pool(name="ps_ca", bufs=2, space="PSUM") as ps_ca:
                    for f in range(KC):
                        pq = ps_ca.tile([P, R], F32, tag="pca", name="pcq")
                        for k in range(KC):
                            nc.tensor.matmul(pq[:], cqt[:, k, bass.ts(f, P)],
                                             x2T[:, k, :],
                                             start=(k == 0), stop=(k == KC - 1))
                        nc.scalar.activation(qcaT[:, f, :], pq[:], AFT.Copy,
                                             scale=0.125)

                attnCT = cap.tile([P, HP, R], BF16, tag="attnCT")
                outt = cap.tile([P, 2, C], F32, tag="outt")
                with tc.tile_pool(name="ps_cs", bufs=2, space="PSUM") as ps_cs, \
                     tc.tile_pool(name="ps_cav", bufs=2, space="PSUM") as ps_cav, \
                     tc.tile_pool(name="ps_crs", bufs=2, space="PSUM") as ps_crs, \
                     tc.tile_pool(name="ps_cpb", bufs=2, space="PSUM") as ps_cpb, \
                     tc.tile_pool(name="expc", bufs=3) as expc:
                    cests, cpavs, cpbs, crss = {}, {}, {}, {}

                    def ca_scores(hp):
                        estc = expc.tile([NTP, 2, R], BF16, tag="estc",
                                         name=f"estc{hp}")
                        nc.gpsimd.memset(estc[:, :, :], 0.0)
                        psc = ps_cs.tile([P, 2, R], F32, tag="pcs")
                        for h01 in range(2):
                            nc.tensor.matmul(psc[0:NTP, h01, :],
                                             KcaT[h01 * D:(h01 + 1) * D, hp, :],
                                             qcaT[h01 * D:(h01 + 1) * D, hp, :],
                                             start=True, stop=True)
                        nc.scalar.activation(estc[0:NT, :, :], psc[0:NT, :, :],
                                             AFT.Exp)
                        cests[hp] = estc

                    def ca_attnv(hp):
                        estc = cests[hp]
                        # h0 -> partitions 0:64, h1 -> 64:128 (no shift DMA);
                        # row-sums via the Vca ones-column over both heads
                        pav = ps_cav.tile([P, R], F32, tag="pcav",
                                          name=f"cpav{hp}")
                        nc.tensor.matmul(pav[0:D, :], Vca[:, 2 * hp, 0:D],
                                         estc[:, 0, :], start=True, stop=True)
                        nc.tensor.matmul(pav[D:P, :], Vca[:, 2 * hp + 1, 0:D],
                                         estc[:, 1, :], start=True, stop=True)
                        prs = ps_crs.tile([1, 2, R], F32, tag="crsum",
                                          name=f"crsum{hp}")
                        nc.tensor.matmul(
                            prs[:].rearrange("p a b -> p (a b)"),
                            Vca[:, 0, D:D + 1],
                            estc[:, :, :].rearrange("p a b -> p (a b)"),
                            start=True, stop=True)
                        rs = tmp.tile([1, 2, R], F32R, tag="crs",
                                      name=f"crs{hp}")
                        with nc.allow_low_precision(reason="f32r softmax recip"):
                            nc.vector.reciprocal(
                                rs[:].rearrange("p a b -> p (a b)"),
                                prs[:].rearrange("p a b -> p (a b)"))
                        cpavs[hp] = pav
                        crss[hp] = rs

                    def ca_bcast(hp):
                        pb = ps_cpb.tile([P, 2 * R], F32, tag="cpb",
                                         name=f"cpb{hp}")
                        nc.tensor.matmul(pb[:], ones_r[0:1, :],
                                         crss[hp][:].rearrange("p a b -> p (a b)"),
                                         start=True, stop=True)
                        pbs_sb = tmp.tile([P, 2 * R], F32, tag="cpbs",
                                          name=f"cpbs{hp}")
                        nc.vector.tensor_copy(pbs_sb[:], pb[:])
                        cpbs[hp] = pbs_sb

                    def ca_divide(hp):
                        pav, pb = cpavs[hp], cpbs[hp]
                        nc.vector.tensor_mul(attnCT[0:D, hp, :], pav[0:D, :],
                                             pb[0:D, 0:R])
                        nc.vector.tensor_mul(attnCT[D:P, hp, :], pav[D:P, :],
                                             pb[D:P, R:2 * R])

                    ca_scores(0)
                    ca_scores(1)
                    ca_attnv(0)
                    for hp in range(2, HP):
                        ca_scores(hp)
                        ca_bcast(hp - 2)
                        ca_attnv(hp - 1)
                        ca_divide(hp - 2)
                    ca_bcast(HP - 2)
                    ca_attnv(HP - 1)
                    ca_divide(HP - 2)
                    ca_bcast(HP - 1)
                    ca_divide(HP - 1)

                # CA O-proj + bias + residual -> out
                with tc.tile_pool(name="ps_co", bufs=2, space="PSUM") as ps_co:
                    for qc in range(2):
                        for f0, fw in ((0, 512), (512, 256)):
                            po = ps_co.tile([P, 512], F32, tag="pco")
                            for hp in range(HP):
                                nc.tensor.matmul(po[:, 0:fw],
                                                 attnCT[:, hp, bass.ts(qc, P)],
                                                 cot[:, hp, f0:f0 + fw],
                                                 start=(hp == 0),
                                                 stop=(hp == HP - 1))
                            nc.vector.tensor_add(outt[:, qc, f0:f0 + fw],
                                                 po[:, 0:fw],
                                                 x2c[:, qc, f0:f0 + fw])
                            nc.sync.dma_start(
                                out_d[qc * P:(qc + 1) * P, f0:f0 + fw],
                                outt[:, qc, f0:f0 + fw])

    nc.compile()
    return nc


def _pack_inputs(inputs):
    """Host-side packing: bf16 weight blobs in SBUF layout + per-core x."""
    import ml_dtypes
    bf16 = ml_dtypes.bfloat16
    f32 = lambda a: np.ascontiguousarray(np.asarray(a), dtype=np.float32)

    def kof(w):   # [768, F] -> [128, 6, F] bf16  ((ko p) f -> p ko f)
        w = f32(w)
        return np.ascontiguousarray(
            w.reshape(KC, P, w.shape[1]).transpose(1, 0, 2).astype(bf16))

    common = {
        "wv": kof(inputs["sa_wv"]),
        "wk": kof(inputs["sa_wk"]),
        "wq": kof(inputs["sa_wq"]),
        "ck": kof(inputs["ca_wk"]),
        "cv": kof(inputs["ca_wv"]),
        "wo": np.ascontiguousarray(
            np.asarray(inputs["sa_wo"], np.float32).reshape(H, D, C)
            .transpose(1, 0, 2).astype(bf16)),
        "cq": kof(inputs["ca_wq"]),
        "co": kof(inputs["ca_wo"]),
    }
    # w1 [768, 6144] -> [p, fc(12), ko(6), ag(2), 256]
    w1 = f32(inputs["ff_w1"]).reshape(KC, P, 2, 12, 256)
    common["w1"] = np.ascontiguousarray(w1.transpose(1, 3, 0, 2, 4).astype(bf16))
    # w2 [3072, 768] -> [p, kq(24), 768]
    w2 = f32(inputs["ff_w2"]).reshape(24, P, C)
    common["w2"] = np.ascontiguousarray(w2.transpose(1, 0, 2).astype(bf16))
    # packed LN vectors (transposed form): {ln1_g, ln1_b, ff_ln_g, ff_ln_b}
    lnvT = np.stack([f32(inputs[k]) for k in
                     ("ln1_g", "ln1_b", "ff_ln_g", "ff_ln_b")], axis=-1)
    common["lnvT"] = np.ascontiguousarray(lnvT.reshape(KC, P, 4).transpose(1, 0, 2))
    # broadcast vectors: {ln2_g, ln2_b, sa_wo_b, ca_wo_b}
    bc = np.stack([f32(inputs[k]) for k in
                   ("ln2_g", "ln2_b", "sa_wo_b", "ca_wo_b")], axis=0)
    common["bcast"] = np.ascontiguousarray(np.broadcast_to(bc[None], (P, 4, C)))
    common["alph"] = np.array([[np.float32(inputs["alpha_attn"]),
                                np.float32(inputs["alpha_dense"])]], np.float32)

    hs = f32(inputs["hidden_states"])
    ehs = f32(inputs["encoder_hidden_states"])
    in_maps = []
    for c in range(8):
        b, r = c // 4, c % 4
        m = dict(common)
        # own rows first, then the rest of the batch (order-invariant attn)
        perm = np.r_[r * R:(r + 1) * R, 0:r * R, (r + 1) * R:N]
        xp = hs[b][perm]
        m["x_full"] = np.ascontiguousarray(xp)
        m["xb"] = np.ascontiguousarray(
            xp.reshape(8, P, C).transpose(1, 0, 2).astype(bf16))
        m["face"] = np.ascontiguousarray(ehs[b, NT:L])
        tT = np.zeros((C, NTP), np.float32)
        tT[:, :NT] = ehs[b, :NT].T
        m["ehsT"] = np.ascontiguousarray(
            tT.reshape(KC, P, NTP).transpose(1, 0, 2).astype(bf16))
        in_maps.append(m)
    return in_maps


def kernel(**inputs):
    fast_ln2 = bool(np.all(np.asarray(inputs["ln2_g"]) == 1.0)
                    and np.all(np.asarray(inputs["ln2_b"]) == 0.0))
    key = ("nc", fast_ln2)
    if key not in _cache:
        _cache[key] = build(fast_ln2)
    nc = _cache["nc"] = _cache[key]

    in_maps = _pack_inputs(inputs)
    res = run_bass_kernel_spmd(nc, in_maps, core_ids=list(range(8)))
    _cache["last_res"] = res
    out = np.empty((B, N, C), np.float32)
    for c in range(8):
        b, r = c // 4, c % 4
        out[b, r * R:(r + 1) * R] = res.results[c]["out_own"]
    return out
